# revision 1
# baseline (speedup 1.0000x reference)
"""Trainium2 Bass kernel for AngularMarginLoss (vocab-parallel softmax loss).

Problem: B=2048, D=256, C=100000, scale=30, margin=0.2, eps=1e-6.
  Wn = W / ||W||_row ; cos = clip(emb @ Wn.T, -1, 1)
  num_b = 30*cos(arccos(cos[b, t_b]) + 0.2)
  denom_b = exp(num_b) + sum_{c != t_b} exp(30*cos[b, c])
  loss = -mean(num_b - log(denom_b + 1e-6))

Sharding: tensor-parallel over the class dim C across 8 NeuronCores
(12500 classes/core, padded to 12544; classic vocab-parallel softmax).
Each core:
  - streams its W shard in 1536-class groups, computes row norms
    (VectorE), folds 30/||w|| into an fp8e4m3 copy of W and transposes
    it with TensorE into [D, Cs] layout; W-prep is software-pipelined
    one group ahead of the matmuls;
  - fp8 DoubleRow matmuls (K=256 in one pass) against the fp8 transposed
    embedding -> PSUM [b, c] logits (= 30*cos, norm+scale prefolded into
    W), ScalarE Exp with accumulate -> per-row partial exp-sums;
  - gathers its owned target rows via indirect DMA (interleaved into the
    early groups), computes the target cosine in f32, masks by ownership;
  - AllReduce #1 (groups 0..6 partial sums + target cosines) fires after
    group 6 so its latency and inter-core skew hide under compute;
    AllReduce #2 (last two groups) runs at the tail;
  - every core computes the identical final scalar loss:
    num = 30*(x*cos(m) - sqrt(1-x^2)*sin(m)), x = clip(target cos), and
    loss = -mean(num - ln(exp(num) + full_sum - exp(30*x) + eps)).
rsqrt/sqrt are computed as Exp/Ln combinations and the activation tables
are pinned to natural_log_exp_and_others so ScalarE never reloads
tables mid-kernel (a mid-loop reload costs ~2.7us of stall each).
The big matmul runs in fp8 (rel err ~4e-4 on the final loss, vs the
2e-2 budget); the target-cosine path stays in f32.
"""

import math
import sys

import numpy as np

if "/opt/trn_rl_repo" not in sys.path:
    sys.path.insert(0, "/opt/trn_rl_repo")

import concourse.bass as bass
import concourse.tile as tile
from concourse import bacc, mybir
from concourse.bass_utils import run_bass_kernel_spmd
from concourse.masks import make_identity

FP32 = mybir.dt.float32
BF16 = mybir.dt.bfloat16
INT32 = mybir.dt.int32

N_CORES = 8
SCALE = 30.0
MARGIN = 0.2
EPS = 1e-6
D = 256  # embedding dim (2 partition tiles)
GSZ = 1536  # classes per activation group (3 PSUM banks)
CHUNK = 512  # matmul free dim (1 PSUM bank)

_TABLES_PATCHED = False


def _patch_act_tables():
    """Force every activation fn we use into one table set so bacc never
    inserts mid-kernel ACT_TABLE_LOADs.  Membership edit only -- set ids
    keep their positions."""
    global _TABLES_PATCHED
    if _TABLES_PATCHED:
        return
    import functools

    import concourse.hw_specs as hw_specs

    orig = hw_specs.get_activation_tables
    KEEP = "natural_log_exp_and_others"
    A = mybir.ActivationFunctionType

    @functools.cache
    def patched(arch):
        tabs = {k: set(v) for k, v in orig(arch).items()}
        assert KEEP in tabs
        for name, fns in tabs.items():
            if name != KEEP:
                for f in (A.Exp, A.Ln, A.Copy, A.Identity):
                    fns.discard(f)
        return tabs

    hw_specs.get_activation_tables = patched
    bacc.get_activation_tables = patched
    _TABLES_PATCHED = True


def build(B: int, csv: int, stage: str = "full", mmdt: str = "fp8"):
    """Build the SPMD Bass program.  csv = valid classes per core."""
    assert B % 128 == 0
    n_bt = B // 128  # b tiles
    # class-group sizes: alternate 3-bank / 4-bank PSUM groups so the two
    # psum slots (3+4=7 banks, +1 transpose bank) ping-pong.
    n_full = csv // GSZ
    tail = csv - n_full * GSZ
    assert 0 <= tail <= CHUNK
    sizes = [GSZ] * n_full + ([tail] if tail else [])
    tags = ["psmm"] * len(sizes)
    assert sum(sizes) == csv
    n_groups = len(sizes)
    bases = [sum(sizes[:i]) for i in range(n_groups)]
    ct_counts = [(w + 127) // 128 for w in sizes]
    cs_pad = sum(c * 128 for c in ct_counts)  # padded shard rows
    max_w = max(sizes)
    psA_w = 2048 if any(w > 1536 for w in sizes) else 1536
    ln30 = math.log(SCALE)
    cos_m = math.cos(MARGIN)
    sin_m = math.sin(MARGIN)
    A = mybir.ActivationFunctionType
    O = mybir.AluOpType
    DT_MM = mybir.dt.float8e4 if mmdt == "fp8" else BF16
    dbl = mmdt == "fp8"

    _patch_act_tables()
    nc = bacc.Bacc(
        "TRN2",
        target_bir_lowering=False,
        debug=False,
        num_devices=N_CORES,
    )

    emb_d = nc.declare_dram_parameter("emb", [B, D], FP32, isOutput=False)
    w_d = nc.declare_dram_parameter("W", [cs_pad, D], FP32, isOutput=False)
    tloc_d = nc.declare_dram_parameter("tloc", [B, 1], INT32, isOutput=False)
    own_d = nc.declare_dram_parameter("own", [B, 1], FP32, isOutput=False)
    out_d = nc.declare_dram_parameter("out", [1, 1], FP32, isOutput=True)

    cc_in = nc.dram_tensor("cc_in", [2, 128, n_bt], FP32)
    cc_out = nc.dram_tensor("cc_out", [2, 128, n_bt], FP32, addr_space="Shared")
    cc2_in = nc.dram_tensor("cc2_in", [128, n_bt], FP32)
    cc2_out = nc.dram_tensor("cc2_out", [128, n_bt], FP32, addr_space="Shared")

    with tile.TileContext(nc, num_cores=N_CORES) as tc:
        import contextlib

        with contextlib.ExitStack() as ctx:
            consts = ctx.enter_context(tc.tile_pool(name="consts", bufs=1))
            embf_p = ctx.enter_context(tc.tile_pool(name="embf", bufs=1))
            embt_p = ctx.enter_context(tc.tile_pool(name="embt", bufs=1))
            acc_p = ctx.enter_context(tc.tile_pool(name="acc", bufs=1))
            wf_p = ctx.enter_context(tc.tile_pool(name="wf", bufs=26))
            wn_p = ctx.enter_context(tc.tile_pool(name="wn", bufs=16))
            wsq_p = ctx.enter_context(tc.tile_pool(name="wsq", bufs=4))
            nrm_p = ctx.enter_context(tc.tile_pool(name="nrm", bufs=3))
            wtg_p = ctx.enter_context(tc.tile_pool(name="wtg", bufs=3))
            exp_p = ctx.enter_context(tc.tile_pool(name="expd", bufs=2))
            tgt_p = ctx.enter_context(tc.tile_pool(name="tgt", bufs=6))
            fin_p = ctx.enter_context(tc.tile_pool(name="fin", bufs=1))
            ps_mm = ctx.enter_context(tc.tile_pool(name="psmm", bufs=2, space="PSUM"))
            ps_tr = ctx.enter_context(tc.tile_pool(name="pstr", bufs=2, space="PSUM"))

            # ---- constants ----
            ident = consts.tile([128, 128], BF16)
            make_identity(nc, ident[:])
            ones = consts.tile([128, 1], FP32)
            nc.vector.memset(ones[:], 1.0)
            b_tiny = consts.tile([128, 1], FP32)
            nc.vector.memset(b_tiny[:], 1e-30)
            b_ln30 = consts.tile([128, 1], FP32)
            nc.vector.memset(b_ln30[:], ln30)
            b_one = consts.tile([128, 1], FP32)
            nc.vector.memset(b_one[:], 1.0)
            b_lnssin = consts.tile([128, 1], FP32)
            nc.vector.memset(b_lnssin[:], math.log(SCALE * sin_m))
            b_eps = consts.tile([128, 1], FP32)
            nc.vector.memset(b_eps[:], EPS)

            # ---- embedding: load f32, cast bf16, transpose to [d, b] ----
            def emit_emb():
                embf = embf_p.tile([128, n_bt * D], FP32)  # [b_in_tile, j*D + d]
                emb3o = embf[:].rearrange("p (j d) -> p j d", j=n_bt)
                emb3i = emb_d[:].rearrange("(j p) d -> p j d", p=128)
                qn = min(4, n_bt)
                for q in range(qn):
                    j0, j1 = q * n_bt // qn, (q + 1) * n_bt // qn
                    nc.sync.dma_start(out=emb3o[:, j0:j1], in_=emb3i[:, j0:j1])
                embc = embf_p.tile([128, n_bt * D], BF16)
                nc.vector.tensor_copy(embc[:], embf[:])
                # embt layout: [d_in_tile, dt*B + j*128 + b]
                embt = embt_p.tile([128, 2 * B], DT_MM)
                n_jq = (n_bt + 3) // 4
                for dt in range(2):
                    for jq in range(n_jq):
                        pt = ps_tr.tile([128, 512], BF16, tag="pstr", name="pt")
                        njs = min(4, n_bt - jq * 4)
                        for ji in range(njs):
                            j = jq * 4 + ji
                            nc.tensor.transpose(
                                out=pt[:, ji * 128 : (ji + 1) * 128],
                                in_=embc[:, j * D + dt * 128 : j * D + (dt + 1) * 128],
                                identity=ident[:],
                            )
                        nc.vector.tensor_copy(
                            embt[:, dt * B + jq * 512 : dt * B + jq * 512 + njs * 128],
                            pt[:, : njs * 128],
                        )
                return embf, embt

            # ---- target path helpers (interleaved into the group loop) ----
            do_tgt = stage in ("tcos", "cc", "full")
            dots = acc_p.tile([128, n_bt], FP32)
            tnrm2 = acc_p.tile([128, n_bt], FP32)
            own_all = acc_p.tile([128, n_bt], FP32)

            def emit_tgt(j):
                tl = tgt_p.tile([128, 1], INT32, tag="tl", name="tl")
                nc.sync.dma_start(out=tl[:], in_=tloc_d[j * 128 : (j + 1) * 128, :])
                nc.sync.dma_start(
                    out=own_all[:, j : j + 1],
                    in_=own_d[j * 128 : (j + 1) * 128, :],
                )
                wt = tgt_p.tile([128, D], FP32, tag="wt", name="wt")
                nc.gpsimd.indirect_dma_start(
                    out=wt[:],
                    out_offset=None,
                    in_=w_d[:],
                    in_offset=bass.IndirectOffsetOnAxis(ap=tl[:, :1], axis=0),
                )
                sc1 = wsq_p.tile([128, D], FP32, tag="wsq", name="sc1")
                nc.vector.scalar_tensor_tensor(
                    out=sc1[:],
                    in0=embf[:, j * D : (j + 1) * D],
                    scalar=0.0,
                    in1=wt[:],
                    op0=O.add,
                    op1=O.mult,
                    accum_out=dots[:, j : j + 1],
                )
                sc2 = wsq_p.tile([128, D], FP32, tag="wsq", name="sc2")
                nc.vector.scalar_tensor_tensor(
                    out=sc2[:],
                    in0=wt[:],
                    scalar=0.0,
                    in1=wt[:],
                    op0=O.add,
                    op1=O.mult,
                    accum_out=tnrm2[:, j : j + 1],
                )

            def emit_tcos():
                # tcos = dots * rsqrt(tnrm2) * own   (rsqrt = exp(-0.5 ln))
                tln = fin_p.tile([128, n_bt], FP32, name="tln")
                nc.scalar.activation(tln[:], tnrm2[:], A.Ln, bias=b_tiny[:])
                trn = fin_p.tile([128, n_bt], FP32, name="trn")
                nc.scalar.activation(trn[:], tln[:], A.Exp, scale=-0.5)
                tc_ = fin_p.tile([128, n_bt], FP32, name="tc_")
                nc.vector.tensor_tensor(
                    out=tc_[:], in0=dots[:], in1=trn[:], op=O.mult
                )
                nc.vector.tensor_tensor(
                    out=tc_[:], in0=tc_[:], in1=own_all[:], op=O.mult
                )
                return tc_

            # ---- W-prep for one class group (pipelined one group ahead) ----
            wtgs: dict = {}

            def prep(g):
                n_ct = ct_counts[g]
                base = bases[g]
                gsz_g = sizes[g]
                wfs = []
                nrm2g = nrm_p.tile([128, 16], FP32, name="nrm2g")
                for t in range(n_ct):
                    wf = wf_p.tile([128, D], FP32, tag="wf", name="wf")
                    nc.sync.dma_start(
                        out=wf[:],
                        in_=w_d[base + t * 128 : base + (t + 1) * 128, :],
                    )
                    wfs.append(wf)
                    wq = wsq_p.tile([128, D], FP32, tag="wsq", name="wq")
                    nc.vector.scalar_tensor_tensor(
                        out=wq[:],
                        in0=wf[:],
                        scalar=0.0,
                        in1=wf[:],
                        op0=O.add,
                        op1=O.mult,
                        accum_out=nrm2g[:, t : t + 1],
                    )
                lng = nrm_p.tile([128, 16], FP32, tag="lng", name="lng")
                nc.scalar.activation(
                    lng[:, :n_ct], nrm2g[:, :n_ct], A.Ln, bias=b_tiny[:, :1]
                )
                rng = nrm_p.tile([128, 16], FP32, tag="rng", name="rng")
                # 30 / ||w|| = exp(-0.5*ln(nrm2) + ln(30))
                nc.scalar.activation(
                    rng[:, :n_ct], lng[:, :n_ct], A.Exp, scale=-0.5,
                    bias=b_ln30[:, :1],
                )
                # wtg layout: [d_in_tile, dt*gw_pad + c]
                gw_pad = n_ct * 128
                wtg = wtg_p.tile([128, 2 * max_w], DT_MM, name="wtg")
                n_sc = (n_ct + 3) // 4
                for t in range(n_ct):
                    wn = wn_p.tile([128, D], BF16, tag="wn", name="wn")
                    nc.vector.tensor_scalar(
                        out=wn[:], in0=wfs[t][:], scalar1=rng[:, t : t + 1],
                        scalar2=None, op0=O.mult,
                    )
                    wfs[t] = wn
                for dt in range(2):
                    for sc in range(n_sc):
                        pt = ps_tr.tile([128, 512], BF16, tag="pstr", name="pt")
                        nts = min(4, n_ct - sc * 4)
                        for ti in range(nts):
                            t = sc * 4 + ti
                            nc.tensor.transpose(
                                out=pt[:, ti * 128 : (ti + 1) * 128],
                                in_=wfs[t][:, dt * 128 : (dt + 1) * 128],
                                identity=ident[:],
                            )
                        nc.vector.tensor_copy(
                            wtg[:, dt * gw_pad + sc * 512 : dt * gw_pad + sc * 512 + nts * 128],
                            pt[:, : nts * 128],
                        )
                wtgs[g] = (wtg, gw_pad)

            # ---- main loop ----
            accs = acc_p.tile([128, n_bt * n_groups], FP32)
            tcos = None
            split_g = max(n_groups - 2, 1)
            tgt_groups = min(split_g, 8)
            per = (n_bt + tgt_groups - 1) // tgt_groups if do_tgt else 0
            do_cc = stage in ("cc", "full")
            embf, embt = emit_emb()
            prep(0)
            for g in range(n_groups):
                if g + 1 < n_groups:
                    prep(g + 1)
                if do_cc and g == split_g:
                    # first AllReduce: groups [0, split_g) + target cosine.
                    # Hides the collective latency + inter-core skew under
                    # the remaining groups' compute.
                    s1 = fin_p.tile([128, n_bt], FP32, name="s1")
                    for j in range(n_bt):
                        nc.vector.tensor_reduce(
                            out=s1[:, j : j + 1],
                            in_=accs[:, j * n_groups : j * n_groups + split_g],
                            axis=mybir.AxisListType.X,
                            op=O.add,
                        )
                    nc.sync.dma_start(out=cc_in[0], in_=s1[:])
                    nc.sync.dma_start(out=cc_in[1], in_=tcos[:])
                    nc.gpsimd.collective_compute(
                        "AllReduce",
                        O.add,
                        replica_groups=[list(range(N_CORES))],
                        ins=[cc_in[:]],
                        outs=[cc_out[:]],
                    )
                if do_tgt:
                    lo = g * per
                    for jj in range(lo, min(lo + per, n_bt)):
                        emit_tgt(jj)
                    if lo + per >= n_bt and tcos is None:
                        tcos = emit_tcos()
                gw = sizes[g]
                n_ch = (gw + CHUNK - 1) // CHUNK
                wtg, gw_pad = wtgs.pop(g)
                embt3 = embt[:].rearrange("p (two b) -> p two b", two=2)
                wtg3 = wtg[:, : 2 * gw_pad].rearrange("p (two c) -> p two c", two=2)
                ps_w = psA_w if tags[g] == "psA" else 1536
                for j in range(n_bt):
                    ps = ps_mm.tile([128, ps_w], FP32, tag=tags[g], name="ps")
                    if dbl:
                        for k in range(n_ch):
                            w0 = k * CHUNK
                            w1 = min(gw, w0 + CHUNK)
                            nc.tensor.matmul(
                                out=ps[:, w0:w1],
                                lhsT=embt3[:, :, j * 128 : (j + 1) * 128],
                                rhs=wtg3[:, :, w0:w1],
                                start=True,
                                stop=True,
                                perf_mode=mybir.MatmulPerfMode.DoubleRow,
                            )
                    else:
                        for dt in range(2):
                            for k in range(n_ch):
                                w0 = k * CHUNK
                                w1 = min(gw, w0 + CHUNK)
                                nc.tensor.matmul(
                                    out=ps[:, w0:w1],
                                    lhsT=embt[:, dt * B + j * 128 : dt * B + (j + 1) * 128],
                                    rhs=wtg[:, dt * gw_pad + w0 : dt * gw_pad + w1],
                                    start=(dt == 0),
                                    stop=(dt == 1),
                                )
                    ed = exp_p.tile([128, max_w], BF16, tag="expd", name="ed")
                    nc.scalar.activation(
                        ed[:, :gw], ps[:, :gw], A.Exp,
                        accum_out=accs[:, j * n_groups + g : j * n_groups + g + 1],
                    )

            # ---- local reduction over the remaining groups ----
            s_loc = fin_p.tile([128, n_bt], FP32)
            nrem = n_groups - split_g
            for j in range(n_bt):
                nc.vector.tensor_reduce(
                    out=s_loc[:, j : j + 1],
                    in_=accs[:, j * n_groups + split_g : (j + 1) * n_groups],
                    axis=mybir.AxisListType.X,
                    op=O.add,
                ) if nrem > 1 else nc.vector.tensor_scalar(
                    out=s_loc[:, j : j + 1],
                    in0=accs[:, j * n_groups + split_g : (j + 1) * n_groups],
                    scalar1=1.0, scalar2=None, op0=O.mult,
                )
            if stage == "sums":
                res0 = fin_p.tile([1, 1], FP32)
                nc.vector.tensor_scalar(
                    out=res0[:], in0=s_loc[:1, :1], scalar1=1.0, scalar2=None,
                    op0=O.mult,
                )
                nc.sync.dma_start(out=out_d[:], in_=res0[:])
            elif stage == "tcos":
                res0 = fin_p.tile([1, 1], FP32)
                nc.vector.tensor_scalar(
                    out=res0[:], in0=tcos[:1, :1], scalar1=1.0, scalar2=None,
                    op0=O.mult,
                )
                nc.sync.dma_start(out=out_d[:], in_=res0[:])
            if stage not in ("cc", "full"):
                nc.compile()
                return nc

            # the target-numerator chain depends only on AllReduce #1's
            # results, so it executes while AllReduce #2 is still in flight
            # (cc2_in fires from the sync engine as soon as s_loc is ready,
            # unaffected by these compute ops) — shortens the exposed tail.
            gs1 = fin_p.tile([128, n_bt], FP32)
            gt = fin_p.tile([128, n_bt], FP32)
            nc.sync.dma_start(out=gs1[:], in_=cc_out[0])
            nc.sync.dma_start(out=gt[:], in_=cc_out[1])
            nc.sync.dma_start(out=cc2_in[:], in_=s_loc[:])
            nc.gpsimd.collective_compute(
                "AllReduce",
                O.add,
                replica_groups=[list(range(N_CORES))],
                ins=[cc2_in[:]],
                outs=[cc2_out[:]],
            )
            e_t = fin_p.tile([128, n_bt], FP32)
            nc.scalar.activation(e_t[:], gt[:], A.Exp, scale=SCALE)
            xc = fin_p.tile([128, n_bt], FP32)
            nc.vector.tensor_scalar(
                out=xc[:], in0=gt[:], scalar1=1.0, scalar2=-1.0,
                op0=O.min, op1=O.max,
            )
            sq = fin_p.tile([128, n_bt], FP32)
            nc.vector.tensor_tensor(out=sq[:], in0=xc[:], in1=xc[:], op=O.mult)
            lnu = fin_p.tile([128, n_bt], FP32)
            nc.scalar.activation(lnu[:], sq[:], A.Ln, scale=-1.0, bias=b_one[:])
            s30 = fin_p.tile([128, n_bt], FP32)
            # 30*sin(m)*sqrt(1-sq) = exp(0.5*ln(1-sq) + ln(30*sin_m))
            nc.scalar.activation(s30[:], lnu[:], A.Exp, scale=0.5, bias=b_lnssin[:])
            num = fin_p.tile([128, n_bt], FP32)
            nc.vector.scalar_tensor_tensor(
                out=num[:], in0=xc[:], scalar=SCALE * cos_m, in1=s30[:],
                op0=O.mult, op1=O.subtract,
            )
            e_n = fin_p.tile([128, n_bt], FP32)
            nc.scalar.activation(e_n[:], num[:], A.Exp)

            gs2 = fin_p.tile([128, n_bt], FP32)
            nc.sync.dma_start(out=gs2[:], in_=cc2_out[:])
            gs = fin_p.tile([128, n_bt], FP32)
            nc.vector.tensor_tensor(out=gs[:], in0=gs1[:], in1=gs2[:], op=O.add)
            excl = fin_p.tile([128, n_bt], FP32)
            nc.vector.tensor_tensor(out=excl[:], in0=gs[:], in1=e_t[:], op=O.subtract)
            den = fin_p.tile([128, n_bt], FP32)
            nc.vector.tensor_tensor(out=den[:], in0=e_n[:], in1=excl[:], op=O.add)
            lden = fin_p.tile([128, n_bt], FP32)
            nc.scalar.activation(lden[:], den[:], A.Ln, bias=b_eps[:])
            pb = fin_p.tile([128, n_bt], FP32)
            nc.vector.tensor_tensor(out=pb[:], in0=num[:], in1=lden[:], op=O.subtract)
            red = fin_p.tile([128, 1], FP32)
            nc.vector.tensor_reduce(
                out=red[:], in_=pb[:], axis=mybir.AxisListType.X, op=O.add
            )
            psf = ps_tr.tile([1, 1], FP32, tag="pstr", name="psf")
            nc.tensor.matmul(out=psf[:], lhsT=red[:], rhs=ones[:], start=True, stop=True)
            res = fin_p.tile([1, 1], FP32)
            nc.vector.tensor_scalar(
                out=res[:], in0=psf[:], scalar1=-1.0 / B, scalar2=None, op0=O.mult
            )
            nc.sync.dma_start(out=out_d[:], in_=res[:])

    nc.compile()
    return nc


_CACHE: dict = {}


def _get(B, csv):
    key = (B, csv)
    if key not in _CACHE:
        _CACHE[key] = build(B, csv)
    return _CACHE[key]


def shard_sizes(csv):
    n_full = csv // GSZ
    tail = csv - n_full * GSZ
    sizes = [GSZ] * n_full + ([tail] if tail else [])
    return sizes


def make_in_maps(embedding, W, targets, B, csv):
    sizes = shard_sizes(csv)
    cs_pad = sum(((w + 127) // 128) * 128 for w in sizes)
    emb = np.ascontiguousarray(embedding, dtype=np.float32)
    t64 = np.asarray(targets).astype(np.int64).reshape(-1)
    in_maps = []
    for i in range(N_CORES):
        c0 = i * csv
        wsh = np.zeros((cs_pad, D), dtype=np.float32)
        wsh[:csv] = W[c0 : c0 + csv]
        tloc = np.clip(t64 - c0, 0, csv - 1).astype(np.int32).reshape(B, 1)
        own = ((t64 >= c0) & (t64 < c0 + csv)).astype(np.float32).reshape(B, 1)
        in_maps.append({"emb": emb, "W": wsh, "tloc": tloc, "own": own})
    return in_maps


def kernel(embedding, W, targets):
    B, csv = 2048, 12500
    assert embedding.shape == (B, D) and W.shape == (N_CORES * csv, D)
    nc = _get(B, csv)
    in_maps = make_in_maps(embedding, W, targets, B, csv)
    res = run_bass_kernel_spmd(nc, in_maps, list(range(N_CORES)))
    return np.asarray(res.results[0]["out"][0, 0], dtype=np.float32)

